# revision 38
# baseline (speedup 1.0000x reference)
"""DGCNN decoder kernel for Trainium2 (8 NeuronCores, SPMD + AllReduce).

Pipeline:
  phase A (pc points sharded over cores, tag-pure 128-point tiles):
    s = 2*y.x - |x|^2 (PE)  ->  exact top-20 of 2048 via 3x(max8/max_index/
    match_replace) (DVE)  ->  gather conv1-transformed candidate columns
    V = Wx'@X (gpsimd indirect_copy)  ->  + per-point term U = [We';Wf']@[y;feat]
    -> Lrelu (BN folded, ACT)  ->  conv2/conv3 (PE+ACT)  ->  max over k
    -> segment-max via additive masks  ->  AllReduce-max of (32,8) obj codes.
  phase B (decoder, core i handles (b,o)=divmod(i,4)):
    5 ResNet blocks of 256x256 matmuls over 2048 points, biases folded.
"""
import sys
for _p in ("/opt/trn_rl_repo",):
    if _p not in sys.path:
        sys.path.insert(0, _p)

import numpy as np

B, N, NY, NOBJ = 2, 2048, 4096, 4
DIM, CDIM, H, K, NBLOCKS = 3, 32, 256, 20, 5
BN_EPS = 1e-5
M = B * NY
NSEG = B * NOBJ
NCORE = 8
TILE = 128
BIG = 1e30
F32 = None  # set after mybir import


def _np(x, dt=np.float32):
    return np.asarray(x).astype(dt)


def host_prep(p, pc, feat, node_tag, params):
    """Returns (core_inputs: list of dict name->array, NT)."""
    p = _np(p); pc = _np(pc); feat = _np(feat)
    node_tag = np.asarray(node_tag).astype(np.int64)

    yflat = pc.reshape(M, DIM)
    featflat = feat.reshape(M, CDIM)
    tiles = []
    for g in range(NSEG):
        pts = np.where(node_tag == g)[0]
        if len(pts) == 0:
            continue
        pad = (-len(pts)) % TILE
        pts = np.concatenate([pts, np.repeat(pts[:1], pad)])
        for t in range(len(pts) // TILE):
            tiles.append((pts[t * TILE:(t + 1) * TILE], g))
    NT = (len(tiles) + NCORE - 1) // NCORE
    while len(tiles) < NT * NCORE:
        tiles.append((tiles[0][0], -1))

    rhs_b = np.zeros((B, 4, N), np.float32)
    XT_b = np.zeros((B, DIM, N), np.float32)
    for b in range(B):
        rhs_b[b, 0:3] = 2.0 * p[b].T
        rhs_b[b, 3] = -(p[b] ** 2).sum(1)
        XT_b[b] = p[b].T

    def fold(w, bn):
        s = _np(bn["gamma"]) / np.sqrt(_np(bn["var"]) + BN_EPS)
        return _np(w) * s[:, None], _np(bn["beta"]) - _np(bn["mean"]) * s
    W1, b1 = fold(params["conv1_w"], params["bn1"])
    W2, b2 = fold(params["conv2_w"], params["bn2"])
    W3, b3 = fold(params["conv3_w"], params["bn3"])
    We, Wx, Wf = W1[:, 0:3], W1[:, 3:6], W1[:, 6:38]
    Wx_eff = Wx - We
    Wu = np.concatenate([We, Wf], axis=1)          # (256, 35)

    import ml_dtypes
    bf16 = ml_dtypes.bfloat16
    shared = dict(
        XTb=XT_b,                                   # (2, 3, 2048)
        WxT=np.ascontiguousarray(Wx_eff.T),         # (3, 256)
        WuT=np.ascontiguousarray(Wu.T),             # (35, 256)
        b1c=np.ascontiguousarray(b1.reshape(2, 128).T),   # (128, 2)
        W2T0=np.ascontiguousarray(W2.T[0:128]),
        W2T1=np.ascontiguousarray(W2.T[128:256]),
        b2c=np.ascontiguousarray(b2.reshape(2, 128).T),
        W3T0=np.ascontiguousarray(W3.T[0:128]),
        W3T1=np.ascontiguousarray(W3.T[128:256]),
        b3c=np.ascontiguousarray(b3.reshape(32, 1)),
        fcp_w=_np(params["fc_p"]["w"]),             # (3, 256)
        fcp_b=np.ascontiguousarray(_np(params["fc_p"]["b"]).reshape(2, 128).T),
        fco_w=_np(params["fc_out"]["w"]),           # (256, 1)
        fco_b=_np(params["fc_out"]["b"]).reshape(1, 1),
    )
    carry = np.zeros(H, np.float32)
    for i in range(NBLOCKS):
        blk = params["blocks"][i]
        shared[f"ccw{i}"] = _np(params["fc_c"][i]["w"])               # (32, 256)
        shared[f"ccb{i}"] = np.ascontiguousarray(
            (_np(params["fc_c"][i]["b"]) + carry).reshape(2, 128).T)  # (128, 2)
        shared[f"W0_{i}"] = _np(blk["fc0"]["w"])
        shared[f"b0_{i}"] = np.ascontiguousarray(_np(blk["fc0"]["b"]).reshape(2, 128).T)
        shared[f"W1_{i}"] = _np(blk["fc1"]["w"])
        carry = _np(blk["fc1"]["b"])
    shared["finb"] = np.ascontiguousarray(carry.reshape(2, 128).T)    # (128, 2)

    core_inputs = []
    for c in range(NCORE):
        ct = tiles[c * NT:(c + 1) * NT]
        lhsT = np.zeros((NT, 4, TILE), np.float32)
        rhs = np.zeros((NT, 4, N), np.float32)
        yfT = np.zeros((NT, 35, TILE), np.float32)
        segsel = np.full((NT, 32, NSEG), -BIG, np.float32)
        boff = np.zeros((NT, TILE, K), np.uint16)
        for t, (pts, g) in enumerate(ct):
            b = (g % B) if g >= 0 else 0
            lhsT[t, 0:3] = yflat[pts].T
            lhsT[t, 3] = 1.0
            rhs[t] = rhs_b[b]
            yfT[t, 0:3] = yflat[pts].T
            yfT[t, 3:35] = featflat[pts].T
            if g >= 0:
                segsel[t, :, g] = 0.0
            boff[t, :, :] = b * N
        bq, oq = c // NOBJ, c % NOBJ
        sel = np.zeros((32, NSEG), np.float32)
        sel[:, bq * NOBJ + oq] = 1.0
        # pack [lhsT | rhs] on 4 partitions and [yfT ; segsel] on 67
        lr = np.concatenate([lhsT, rhs], axis=2)          # (NT, 4, 128+N)
        ys = np.zeros((NT, 35, TILE + NSEG), np.float32)
        ys[:, 0:35, 0:TILE] = yfT
        ys[:, 0:32, TILE:TILE + NSEG] = segsel
        ci = dict(lr_d=lr, ys_d=ys,
                  boff_d=boff, sel_d=sel,
                  pbT_d=np.ascontiguousarray(p[bq].T))
        ci.update({k + "_d": v for k, v in shared.items()})
        core_inputs.append(ci)
    return core_inputs, NT


# ------------------------------------------------------------------ program

_PROG_CACHE = {}


def build_program(NT):
    import concourse.bass as bass
    import concourse.bacc as bacc
    import concourse.mybir as mybir
    import concourse.tile as tile

    f32 = mybir.dt.float32
    fr = mybir.dt.float32r
    b16 = mybir.dt.bfloat16
    u16 = mybir.dt.uint16
    AF = mybir.ActivationFunctionType
    OP = mybir.AluOpType
    AX = mybir.AxisListType

    nc = bacc.Bacc(None, target_bir_lowering=False, debug=False,
                   num_devices=NCORE)

    def din(name, shape, dt=f32):
        return nc.dram_tensor(name, list(shape), dt, kind="ExternalInput").ap()

    # phase A per-tile inputs (packed)
    lr_d = din("lr_d", (NT, 4, TILE + N))
    ys_d = din("ys_d", (NT, 35, TILE + NSEG))
    boff_d = din("boff_d", (NT, TILE, K), u16)
    # shared weights
    XTb_d = din("XTb_d", (B, DIM, N))
    WxT_d = din("WxT_d", (DIM, H))
    WuT_d = din("WuT_d", (35, H))
    b1c_d = din("b1c_d", (128, 2))
    W2T0_d = din("W2T0_d", (128, H))
    W2T1_d = din("W2T1_d", (128, H))
    b2c_d = din("b2c_d", (128, 2))
    W3T0_d = din("W3T0_d", (128, CDIM))
    W3T1_d = din("W3T1_d", (128, CDIM))
    b3c_d = din("b3c_d", (CDIM, 1))
    # phase B
    sel_d = din("sel_d", (CDIM, NSEG))
    pbT_d = din("pbT_d", (DIM, N))
    fcp_w_d = din("fcp_w_d", (DIM, H))
    fcp_b_d = din("fcp_b_d", (128, 2))
    ccw_d = [din(f"ccw{i}_d", (CDIM, H)) for i in range(NBLOCKS)]
    ccb_d = [din(f"ccb{i}_d", (128, 2)) for i in range(NBLOCKS)]
    W0_d = [din(f"W0_{i}_d", (H, H)) for i in range(NBLOCKS)]
    b0_d = [din(f"b0_{i}_d", (128, 2)) for i in range(NBLOCKS)]
    W1_d = [din(f"W1_{i}_d", (H, H)) for i in range(NBLOCKS)]
    finb_d = din("finb_d", (128, 2))
    fco_w_d = din("fco_w_d", (H, 1))
    fco_b_d = din("fco_b_d", (1, 1))

    out_y = nc.dram_tensor("out_y", [1, N], f32, kind="ExternalOutput").ap()

    cc_in = nc.dram_tensor("cc_in", [CDIM, NSEG], f32).ap()
    cc_out = nc.dram_tensor("cc_out", [CDIM, NSEG], f32, addr_space="Shared").ap()

    with tile.TileContext(nc) as tc:
        # =========================== phase A ===========================
        with (
            tc.tile_pool(name="constA", bufs=1) as cA,
            tc.tile_pool(name="io", bufs=3) as io,
            tc.tile_pool(name="tk", bufs=2) as tk,
            tc.tile_pool(name="tk1", bufs=2) as tk1,
            tc.tile_pool(name="act", bufs=2) as actp,
            tc.tile_pool(name="act1", bufs=1) as act1,
            tc.tile_pool(name="ps_s", bufs=2, space="PSUM") as ps_s,
            tc.tile_pool(name="ps_u", bufs=1, space="PSUM") as ps_u,
            tc.tile_pool(name="ps_c", bufs=2, space="PSUM") as ps_c,
            tc.tile_pool(name="ps_3", bufs=2, space="PSUM") as ps_3,
            tc.tile_pool(name="dram", bufs=2, space="DRAM") as dr,
        ):
            # ---- load shared constants ----
            XTb = cA.tile([DIM, B * N], f32)
            for b in range(B):
                nc.sync.dma_start(XTb[:, b * N:(b + 1) * N], XTb_d[b])
            WxT = cA.tile([DIM, H], f32)
            nc.sync.dma_start(WxT[:], WxT_d[:])
            WuT = cA.tile([35, H], f32)
            nc.sync.dma_start(WuT[:], WuT_d[:])
            b1c = cA.tile([128, 2], f32)
            nc.sync.dma_start(b1c[:], b1c_d[:])
            W2T0 = cA.tile([128, H], f32)
            nc.sync.dma_start(W2T0[:], W2T0_d[:])
            W2T1 = cA.tile([128, H], f32)
            nc.sync.dma_start(W2T1[:], W2T1_d[:])
            b2c = cA.tile([128, 2], f32)
            nc.sync.dma_start(b2c[:], b2c_d[:])
            W3T0 = cA.tile([128, CDIM], f32)
            nc.sync.dma_start(W3T0[:], W3T0_d[:])
            W3T1 = cA.tile([128, CDIM], f32)
            nc.sync.dma_start(W3T1[:], W3T1_d[:])
            b3c = cA.tile([CDIM, 1], f32)
            nc.sync.dma_start(b3c[:], b3c_d[:])

            # ---- V = WxT.T @ X per batch: VV[j] (128, B*N) ----
            VV = [cA.tile([128, B * N], f32, tag=f"VV{j}", name=f"VV{j}")
                  for j in range(2)]
            for j in range(2):
                for b in range(B):
                    for n4 in range(N // 512):
                        pv = ps_s.tile([128, 512], f32, tag="ps")
                        nc.tensor.matmul(
                            pv[:], WxT[:, j * 128:(j + 1) * 128],
                            XTb[:, b * N + n4 * 512: b * N + (n4 + 1) * 512],
                            start=True, stop=True)
                        nc.scalar.copy(
                            VV[j][:, b * N + n4 * 512: b * N + (n4 + 1) * 512],
                            pv[:])

            stage = cA.tile([CDIM, NSEG * NT], f32)

            SC = TILE * K          # columns per tile (k-major: col = k*128+pt)
            for t in range(NT):
                # ---- per-tile loads (packed: [lhsT|rhs], [yfT;segsel]) ----
                lr = io.tile([4, TILE + N], f32, tag="lr")
                nc.sync.dma_start(lr[:], lr_d[t])
                ys = io.tile([35, TILE + NSEG], f32, tag="ys")
                nc.sync.dma_start(ys[:], ys_d[t])
                boff = io.tile([TILE, K], u16, tag="boff")
                nc.sync.dma_start(boff[:], boff_d[t])
                lhsT = lr[:, 0:TILE]
                yfT = ys[0:35, 0:TILE]
                segsel = ys[0:32, TILE:TILE + NSEG]

                # ---- s = lhsT.T @ rhs ----
                sv0 = tk.tile([TILE, N], f32, tag="sv0")
                for n4 in range(N // 512):
                    ps = ps_s.tile([TILE, 512], f32, tag="ps")
                    nc.tensor.matmul(ps[:], lhsT,
                                     lr[:, TILE + n4 * 512:TILE + (n4 + 1) * 512],
                                     start=True, stop=True)
                    nc.scalar.copy(sv0[:, n4 * 512:(n4 + 1) * 512], ps[:])

                # ---- top-20 (3 rounds of top-8) ----
                idx = tk1.tile([TILE, 24], u16, tag="idx")
                m8 = tk1.tile([TILE, 8], f32, tag="m8")
                sv1 = tk1.tile([TILE, N], f32, tag="sv1")
                sv2 = tk1.tile([TILE, N], f32, tag="sv2")
                nc.vector.max(m8[:], sv0[:])
                nc.vector.max_index(idx[:, 0:8], m8[:], sv0[:])
                nc.vector.match_replace(sv1[:], m8[:], sv0[:], -BIG)
                m8b = tk1.tile([TILE, 8], f32, tag="m8b")
                nc.vector.max(m8b[:], sv1[:])
                nc.vector.max_index(idx[:, 8:16], m8b[:], sv1[:])
                nc.vector.match_replace(sv2[:], m8b[:], sv1[:], -BIG)
                m8c = tk1.tile([TILE, 8], f32, tag="m8c")
                nc.vector.max(m8c[:], sv2[:])
                nc.vector.max_index(idx[:, 16:24], m8c[:], sv2[:])

                idxo = tk1.tile([TILE, K], u16, tag="idxo")
                nc.vector.tensor_tensor(idxo[:], idx[:, 0:K], boff[:], op=OP.add)

                # ---- wrap indices for the 16-partition-group gather ----
                # column order j = 320q + 16k + p16  <->  (pt = 16q+p16, k);
                # wrapped layout W128[16g+p16, 20q+k] = idx[16q+p16, k],
                # replicated to all 8 groups via a step-0 broadcast dim.
                scr = dr.tile([TILE, K], u16)
                nc.sync.dma_start(scr[:], idxo[:])
                wrapv = scr[:].flatten().rearrange("(q p k) -> p q k",
                                                   q=8, p=16, k=K)
                W128 = tk1.tile([128, K * 8], u16, tag="W128")
                for g in range(8):
                    nc.sync.dma_start(
                        W128[16 * g:16 * (g + 1), :].rearrange(
                            "p (q k) -> p q k", q=8),
                        wrapv)

                # ---- U = WuT.T @ [y;feat] ----
                Usb = []
                for j in range(2):
                    pu = ps_u.tile([128, TILE], f32)
                    nc.tensor.matmul(pu[:], WuT[:, j * 128:(j + 1) * 128],
                                     yfT[:], start=True, stop=True)
                    u = actp.tile([128, TILE], f32, tag=f"U{j}")
                    nc.scalar.copy(u[:], pu[:])
                    Usb.append(u)

                # ---- gather V columns + add U + Lrelu -> a1 (in place) ----
                a1 = []
                for j in range(2):
                    G = tk.tile([128, SC], f32, tag=f"G{j}")
                    nc.gpsimd.ap_gather(G[:], VV[j][:],
                                        W128[:].bitcast(mybir.dt.int16),
                                        channels=128, num_elems=B * N, d=1,
                                        num_idxs=SC)
                    nc.vector.tensor_tensor(
                        G[:].rearrange("p (q k s) -> p q k s", q=8, k=K),
                        G[:].rearrange("p (q k s) -> p q k s", q=8, k=K),
                        Usb[j][:].rearrange("p (q s) -> p q s", q=8)
                        .unsqueeze(2).broadcast_to((128, 8, K, 16)),
                        op=OP.add)
                    nc.scalar.activation(G[:], G[:], AF.Prelu,
                                         bias=b1c[:, j:j + 1], scale=1.0,
                                         alpha=0.2)
                    a1.append(G)

                # ---- conv2 ----
                a2 = []
                for i2 in range(2):
                    a = act1.tile([128, SC], f32, tag=f"a2_{i2}")
                    for c5 in range(SC // 512):
                        pc_ = ps_c.tile([128, 512], f32)
                        sl = slice(c5 * 512, (c5 + 1) * 512)
                        nc.tensor.matmul(pc_[:], W2T0[:, i2 * 128:(i2 + 1) * 128],
                                         a1[0][:, sl], start=True, stop=False)
                        nc.tensor.matmul(pc_[:], W2T1[:, i2 * 128:(i2 + 1) * 128],
                                         a1[1][:, sl], start=False, stop=True)
                        nc.scalar.activation(a[:, sl], pc_[:], AF.Prelu,
                                             bias=b2c[:, i2:i2 + 1], scale=1.0,
                                             alpha=0.2)
                    a2.append(a)

                # ---- conv3 ----
                a3 = act1.tile([CDIM, SC], f32, tag="a3")
                for c5 in range(SC // 512):
                    p3 = ps_3.tile([CDIM, 512], f32, tag="p3")
                    sl = slice(c5 * 512, (c5 + 1) * 512)
                    nc.tensor.matmul(p3[:], W3T0[:], a2[0][:, sl],
                                     start=True, stop=False)
                    nc.tensor.matmul(p3[:], W3T1[:], a2[1][:, sl],
                                     start=False, stop=True)
                    nc.scalar.activation(a3[:, sl], p3[:], AF.Prelu,
                                         bias=b3c[:], scale=1.0, alpha=0.2)

                # ---- max over k, then segment stage ----
                cmax = tk1.tile([CDIM, TILE], f32, tag="cmax")
                nc.vector.reduce_max(
                    cmax[:].rearrange("p (q s) -> p q s", q=8),
                    a3[:].rearrange("p (q k s) -> p q s k", q=8, k=K),
                    axis=AX.X)
                tmax = tk1.tile([CDIM, 1], f32, tag="tmax")
                nc.vector.reduce_max(tmax[:], cmax[:], axis=AX.X)
                nc.vector.tensor_tensor(
                    stage[:, t * NSEG:(t + 1) * NSEG],
                    tmax[:].broadcast_to((CDIM, NSEG)),
                    segsel[:], op=OP.add)

            # ---- reduce stage -> obj_local, allreduce ----
            obj_local = cA.tile([CDIM, NSEG], f32)
            nc.vector.reduce_max(
                obj_local[:],
                stage[:].rearrange("p (t s) -> p s t", s=NSEG),
                axis=AX.X)
            nc.sync.dma_start(cc_in[:], obj_local[:])
            nc.gpsimd.collective_compute(
                "AllReduce", OP.max,
                replica_groups=[list(range(NCORE))],
                ins=[cc_in[:]], outs=[cc_out[:]])

        # =========================== phase B ===========================
        with (
            tc.tile_pool(name="constB", bufs=1) as cB,
            tc.tile_pool(name="net", bufs=2) as netp,
            tc.tile_pool(name="actB", bufs=2) as actB,
            tc.tile_pool(name="ps_b", bufs=4, space="PSUM") as ps_b,
            tc.tile_pool(name="ps_cc", bufs=2, space="PSUM") as ps_cc,
        ):
            objT = cB.tile([CDIM, NSEG], f32)
            nc.sync.dma_start(objT[:], cc_out[:])
            sel = cB.tile([CDIM, NSEG], f32)
            nc.sync.dma_start(sel[:], sel_d[:])
            tmp8 = cB.tile([CDIM, NSEG], f32)
            nc.vector.tensor_mul(tmp8[:], objT[:], sel[:])
            obj_sel = cB.tile([CDIM, 1], f32)
            nc.vector.reduce_sum(obj_sel[:], tmp8[:], axis=mybir.AxisListType.X)

            pbT = cB.tile([DIM, N], f32)
            nc.sync.dma_start(pbT[:], pbT_d[:])
            fcp_w = cB.tile([DIM, H], f32)
            nc.sync.dma_start(fcp_w[:], fcp_w_d[:])
            fcp_b = cB.tile([128, 2], f32)
            nc.sync.dma_start(fcp_b[:], fcp_b_d[:])
            finb = cB.tile([128, 2], f32)
            nc.sync.dma_start(finb[:], finb_d[:])
            fco_w = cB.tile([128, 2], f32)
            for j in range(2):
                nc.sync.dma_start(fco_w[:, j:j + 1], fco_w_d[j * 128:(j + 1) * 128])
            fco_b = cB.tile([1, 1], f32)
            nc.sync.dma_start(fco_b[:], fco_b_d[:])

            ccw = [cB.tile([CDIM, H], f32, name=f"ccw{i}") for i in range(NBLOCKS)]
            ccb = [cB.tile([128, 2], f32, name=f"ccb{i}") for i in range(NBLOCKS)]
            W0 = [[cB.tile([128, H], f32, name=f"W0_{i}_{j}") for j in range(2)]
                  for i in range(NBLOCKS)]
            b0 = [cB.tile([128, 2], f32, name=f"b0_{i}") for i in range(NBLOCKS)]
            W1 = [[cB.tile([128, H], f32, name=f"W1_{i}_{j}") for j in range(2)]
                  for i in range(NBLOCKS)]
            for i in range(NBLOCKS):
                nc.sync.dma_start(ccw[i][:], ccw_d[i][:])
                nc.sync.dma_start(ccb[i][:], ccb_d[i][:])
                nc.sync.dma_start(b0[i][:], b0_d[i][:])
                for j in range(2):
                    nc.sync.dma_start(W0[i][j][:], W0_d[i][j * 128:(j + 1) * 128])
                    nc.sync.dma_start(W1[i][j][:], W1_d[i][j * 128:(j + 1) * 128])

            NB4 = N // 512
            # net0
            net = []
            for j in range(2):
                nt_ = netp.tile([128, N], f32, tag=f"net{j}")
                for n4 in range(NB4):
                    pb = ps_b.tile([128, 512], f32)
                    nc.tensor.matmul(pb[:], fcp_w[:, j * 128:(j + 1) * 128],
                                     pbT[:, n4 * 512:(n4 + 1) * 512],
                                     start=True, stop=True)
                    nc.scalar.activation(nt_[:, n4 * 512:(n4 + 1) * 512], pb[:],
                                         AF.Identity, bias=fcp_b[:, j:j + 1])
                net.append(nt_)

            # W0/W1 lhsT slices: W (256, 256) rows = in-ch (K dim).
            for i in range(NBLOCKS):
                ccv = []
                for j in range(2):
                    pcc = ps_cc.tile([128, 1], f32)
                    nc.tensor.matmul(pcc[:], ccw[i][:, j * 128:(j + 1) * 128],
                                     obj_sel[:], start=True, stop=True)
                    cv = actB.tile([128, 1], f32, tag=f"ccv{j}")
                    nc.scalar.activation(cv[:], pcc[:], AF.Identity,
                                         bias=ccb[i][:, j:j + 1])
                    ccv.append(cv)
                net1, rnet = [], []
                for j in range(2):
                    n1 = netp.tile([128, N], f32, tag=f"net1_{j}")
                    nc.vector.tensor_scalar_add(n1[:], net[j][:], ccv[j][:])
                    net1.append(n1)
                    rn = actB.tile([128, N], f32, tag=f"rnet{j}")
                    nc.scalar.activation(rn[:], net[j][:], AF.Relu,
                                         bias=ccv[j][:])
                    rnet.append(rn)
                rh = []
                for i2 in range(2):
                    r = actB.tile([128, N], f32, tag=f"rh{i2}")
                    for n4 in range(NB4):
                        pb = ps_b.tile([128, 512], f32)
                        sl = slice(n4 * 512, (n4 + 1) * 512)
                        nc.tensor.matmul(pb[:], W0[i][0][:, i2 * 128:(i2 + 1) * 128],
                                         rnet[0][:, sl], start=True, stop=False)
                        nc.tensor.matmul(pb[:], W0[i][1][:, i2 * 128:(i2 + 1) * 128],
                                         rnet[1][:, sl], start=False, stop=True)
                        nc.scalar.activation(r[:, sl], pb[:], AF.Relu,
                                             bias=b0[i][:, i2:i2 + 1])
                    rh.append(r)
                net_next = []
                for i2 in range(2):
                    n2 = netp.tile([128, N], f32, tag=f"net{i2}")
                    for n4 in range(NB4):
                        pb = ps_b.tile([128, 512], f32)
                        sl = slice(n4 * 512, (n4 + 1) * 512)
                        nc.tensor.matmul(pb[:], W1[i][0][:, i2 * 128:(i2 + 1) * 128],
                                         rh[0][:, sl], start=True, stop=False)
                        nc.tensor.matmul(pb[:], W1[i][1][:, i2 * 128:(i2 + 1) * 128],
                                         rh[1][:, sl], start=False, stop=True)
                        nc.vector.tensor_tensor(n2[:, sl], net1[i2][:, sl],
                                                pb[:], op=OP.add)
                    net_next.append(n2)
                net = net_next

            # final relu + fc_out
            rfin = []
            for j in range(2):
                r = actB.tile([128, N], f32, tag=f"rfin{j}")
                nc.scalar.activation(r[:], net[j][:], AF.Relu,
                                     bias=finb[:, j:j + 1])
                rfin.append(r)
            outv = cB.tile([1, N], f32)
            for n4 in range(NB4):
                po = ps_cc.tile([1, 512], f32, tag="po")
                sl = slice(n4 * 512, (n4 + 1) * 512)
                nc.tensor.matmul(po[:], fco_w[:, 0:1], rfin[0][:, sl],
                                 start=True, stop=False)
                nc.tensor.matmul(po[:], fco_w[:, 1:2], rfin[1][:, sl],
                                 start=False, stop=True)
                nc.scalar.activation(outv[:, sl], po[:], AF.Identity,
                                     bias=fco_b[:])
            nc.sync.dma_start(out_y[:], outv[:])

    nc.compile()
    return nc


def get_program(NT):
    if NT not in _PROG_CACHE:
        _PROG_CACHE[NT] = build_program(NT)
    return _PROG_CACHE[NT]


def kernel(p, pc, feat, node_tag, params):
    from concourse.bass_utils import run_bass_kernel_spmd
    core_inputs, NT = host_prep(p, pc, feat, node_tag, params)
    nc = get_program(NT)
    res = run_bass_kernel_spmd(nc, core_inputs, list(range(NCORE)))
    out = np.zeros((B, NOBJ, N), np.float32)
    for c in range(NCORE):
        out[c // NOBJ, c % NOBJ] = res.results[c]["out_y"][0]
    return out


# revision 40
# speedup vs baseline: 1.0806x; 1.0806x over previous
"""DGCNN decoder kernel for Trainium2 (8 NeuronCores, SPMD + AllReduce).

Pipeline:
  phase A (pc points sharded over cores, tag-pure 128-point tiles):
    s = 2*y.x - |x|^2 (PE)  ->  exact top-20 of 2048 via 3x(max8/max_index/
    match_replace) (DVE)  ->  gather conv1-transformed candidate columns
    V = Wx'@X (gpsimd indirect_copy)  ->  + per-point term U = [We';Wf']@[y;feat]
    -> Lrelu (BN folded, ACT)  ->  conv2/conv3 (PE+ACT)  ->  max over k
    -> segment-max via additive masks  ->  AllReduce-max of (32,8) obj codes.
  phase B (decoder, core i handles (b,o)=divmod(i,4)):
    5 ResNet blocks of 256x256 matmuls over 2048 points, biases folded.
"""
import sys
for _p in ("/opt/trn_rl_repo",):
    if _p not in sys.path:
        sys.path.insert(0, _p)

import numpy as np

B, N, NY, NOBJ = 2, 2048, 4096, 4
DIM, CDIM, H, K, NBLOCKS = 3, 32, 256, 20, 5
BN_EPS = 1e-5
M = B * NY
NSEG = B * NOBJ
NCORE = 8
TILE = 128
BIG = 1e30
F32 = None  # set after mybir import


def _np(x, dt=np.float32):
    return np.asarray(x).astype(dt)


def host_prep(p, pc, feat, node_tag, params):
    """Returns (core_inputs: list of dict name->array, NT)."""
    p = _np(p); pc = _np(pc); feat = _np(feat)
    node_tag = np.asarray(node_tag).astype(np.int64)

    yflat = pc.reshape(M, DIM)
    featflat = feat.reshape(M, CDIM)
    tiles = []
    for g in range(NSEG):
        pts = np.where(node_tag == g)[0]
        if len(pts) == 0:
            continue
        pad = (-len(pts)) % TILE
        pts = np.concatenate([pts, np.repeat(pts[:1], pad)])
        for t in range(len(pts) // TILE):
            tiles.append((pts[t * TILE:(t + 1) * TILE], g))
    NT = (len(tiles) + NCORE - 1) // NCORE
    while len(tiles) < NT * NCORE:
        tiles.append((tiles[0][0], -1))

    rhs_b = np.zeros((B, 4, N), np.float32)
    XT_b = np.zeros((B, DIM, N), np.float32)
    for b in range(B):
        rhs_b[b, 0:3] = 2.0 * p[b].T
        rhs_b[b, 3] = -(p[b] ** 2).sum(1)
        XT_b[b] = p[b].T

    def fold(w, bn):
        s = _np(bn["gamma"]) / np.sqrt(_np(bn["var"]) + BN_EPS)
        return _np(w) * s[:, None], _np(bn["beta"]) - _np(bn["mean"]) * s
    W1, b1 = fold(params["conv1_w"], params["bn1"])
    W2, b2 = fold(params["conv2_w"], params["bn2"])
    W3, b3 = fold(params["conv3_w"], params["bn3"])
    We, Wx, Wf = W1[:, 0:3], W1[:, 3:6], W1[:, 6:38]
    Wx_eff = Wx - We
    Wu = np.concatenate([We, Wf], axis=1)          # (256, 35)

    import ml_dtypes
    bf16 = ml_dtypes.bfloat16
    shared = dict(
        XTb=XT_b,                                   # (2, 3, 2048)
        WxT=np.ascontiguousarray(Wx_eff.T),         # (3, 256)
        WuT=np.ascontiguousarray(Wu.T),             # (35, 256)
        b1c=np.ascontiguousarray(b1.reshape(2, 128).T),   # (128, 2)
        W2T0=np.ascontiguousarray(W2.T[0:128]),
        W2T1=np.ascontiguousarray(W2.T[128:256]),
        b2c=np.ascontiguousarray(b2.reshape(2, 128).T),
        W3T0=np.ascontiguousarray(W3.T[0:128]),
        W3T1=np.ascontiguousarray(W3.T[128:256]),
        b3c=np.ascontiguousarray(b3.reshape(32, 1)),
        fcp_w=_np(params["fc_p"]["w"]),             # (3, 256)
        fcp_b=np.ascontiguousarray(_np(params["fc_p"]["b"]).reshape(2, 128).T),
        fco_w=_np(params["fc_out"]["w"]),           # (256, 1)
        fco_b=_np(params["fc_out"]["b"]).reshape(1, 1),
    )
    carry = np.zeros(H, np.float32)
    for i in range(NBLOCKS):
        blk = params["blocks"][i]
        shared[f"ccw{i}"] = _np(params["fc_c"][i]["w"])               # (32, 256)
        shared[f"ccb{i}"] = np.ascontiguousarray(
            (_np(params["fc_c"][i]["b"]) + carry).reshape(2, 128).T)  # (128, 2)
        shared[f"W0_{i}"] = _np(blk["fc0"]["w"])
        shared[f"b0_{i}"] = np.ascontiguousarray(_np(blk["fc0"]["b"]).reshape(2, 128).T)
        shared[f"W1_{i}"] = _np(blk["fc1"]["w"])
        carry = _np(blk["fc1"]["b"])
    shared["finb"] = np.ascontiguousarray(carry.reshape(2, 128).T)    # (128, 2)

    core_inputs = []
    for c in range(NCORE):
        ct = tiles[c * NT:(c + 1) * NT]
        lhsT = np.zeros((NT, 4, TILE), np.float32)
        rhs = np.zeros((NT, 4, N), np.float32)
        yfT = np.zeros((NT, 35, TILE), np.float32)
        segsel = np.full((NT, 32, NSEG), -BIG, np.float32)
        boff = np.zeros((NT, TILE, K), np.uint16)
        for t, (pts, g) in enumerate(ct):
            b = (g % B) if g >= 0 else 0
            lhsT[t, 0:3] = yflat[pts].T
            lhsT[t, 3] = 1.0
            rhs[t] = rhs_b[b]
            yfT[t, 0:3] = yflat[pts].T
            yfT[t, 3:35] = featflat[pts].T
            if g >= 0:
                segsel[t, :, g] = 0.0
            boff[t, :, :] = b * N
        bq, oq = c // NOBJ, c % NOBJ
        sel = np.zeros((32, NSEG), np.float32)
        sel[:, bq * NOBJ + oq] = 1.0
        # pack [lhsT | rhs] on 4 partitions and [yfT ; segsel] on 67
        lr = np.concatenate([lhsT, rhs], axis=2)          # (NT, 4, 128+N)
        ys = np.zeros((NT, 35, TILE + NSEG), np.float32)
        ys[:, 0:35, 0:TILE] = yfT
        ys[:, 0:32, TILE:TILE + NSEG] = segsel
        ci = dict(lr_d=lr, ys_d=ys,
                  boff_d=boff, sel_d=sel,
                  pbT_d=np.ascontiguousarray(p[bq].T))
        ci.update({k + "_d": v for k, v in shared.items()})
        core_inputs.append(ci)
    return core_inputs, NT


# ------------------------------------------------------------------ program

_PROG_CACHE = {}


def build_program(NT):
    import concourse.bass as bass
    import concourse.bacc as bacc
    import concourse.mybir as mybir
    import concourse.tile as tile

    f32 = mybir.dt.float32
    fr = mybir.dt.float32r
    b16 = mybir.dt.bfloat16
    u16 = mybir.dt.uint16
    AF = mybir.ActivationFunctionType
    OP = mybir.AluOpType
    AX = mybir.AxisListType

    nc = bacc.Bacc(None, target_bir_lowering=False, debug=False,
                   num_devices=NCORE)

    def din(name, shape, dt=f32):
        return nc.dram_tensor(name, list(shape), dt, kind="ExternalInput").ap()

    # phase A per-tile inputs (packed)
    lr_d = din("lr_d", (NT, 4, TILE + N))
    ys_d = din("ys_d", (NT, 35, TILE + NSEG))
    boff_d = din("boff_d", (NT, TILE, K), u16)
    # shared weights
    XTb_d = din("XTb_d", (B, DIM, N))
    WxT_d = din("WxT_d", (DIM, H))
    WuT_d = din("WuT_d", (35, H))
    b1c_d = din("b1c_d", (128, 2))
    W2T0_d = din("W2T0_d", (128, H), fr)
    W2T1_d = din("W2T1_d", (128, H), fr)
    b2c_d = din("b2c_d", (128, 2))
    W3T0_d = din("W3T0_d", (128, CDIM), fr)
    W3T1_d = din("W3T1_d", (128, CDIM), fr)
    b3c_d = din("b3c_d", (CDIM, 1))
    # phase B
    sel_d = din("sel_d", (CDIM, NSEG))
    pbT_d = din("pbT_d", (DIM, N))
    fcp_w_d = din("fcp_w_d", (DIM, H))
    fcp_b_d = din("fcp_b_d", (128, 2))
    ccw_d = [din(f"ccw{i}_d", (CDIM, H)) for i in range(NBLOCKS)]
    ccb_d = [din(f"ccb{i}_d", (128, 2)) for i in range(NBLOCKS)]
    W0_d = [din(f"W0_{i}_d", (H, H), fr) for i in range(NBLOCKS)]
    b0_d = [din(f"b0_{i}_d", (128, 2)) for i in range(NBLOCKS)]
    W1_d = [din(f"W1_{i}_d", (H, H), fr) for i in range(NBLOCKS)]
    finb_d = din("finb_d", (128, 2))
    fco_w_d = din("fco_w_d", (H, 1))
    fco_b_d = din("fco_b_d", (1, 1))

    out_y = nc.dram_tensor("out_y", [1, N], f32, kind="ExternalOutput").ap()

    cc_in = nc.dram_tensor("cc_in", [CDIM, NSEG], f32).ap()
    cc_out = nc.dram_tensor("cc_out", [CDIM, NSEG], f32, addr_space="Shared").ap()

    with tile.TileContext(nc) as tc:
        # =========================== phase A ===========================
        with (
            tc.tile_pool(name="constA", bufs=1) as cA,
            tc.tile_pool(name="io", bufs=3) as io,
            tc.tile_pool(name="tk", bufs=2) as tk,
            tc.tile_pool(name="tk1", bufs=1) as tk1,
            tc.tile_pool(name="act", bufs=2) as actp,
            tc.tile_pool(name="act1", bufs=1) as act1,
            tc.tile_pool(name="ps_s", bufs=2, space="PSUM") as ps_s,
            tc.tile_pool(name="ps_u", bufs=1, space="PSUM") as ps_u,
            tc.tile_pool(name="ps_c", bufs=2, space="PSUM") as ps_c,
            tc.tile_pool(name="ps_3", bufs=2, space="PSUM") as ps_3,
            tc.tile_pool(name="dram", bufs=2, space="DRAM") as dr,
        ):
            # ---- load shared constants ----
            XTb = cA.tile([DIM, B * N], f32)
            for b in range(B):
                nc.sync.dma_start(XTb[:, b * N:(b + 1) * N], XTb_d[b])
            WxT = cA.tile([DIM, H], f32)
            nc.sync.dma_start(WxT[:], WxT_d[:])
            WuT = cA.tile([35, H], f32)
            nc.sync.dma_start(WuT[:], WuT_d[:])
            b1c = cA.tile([128, 2], f32)
            nc.sync.dma_start(b1c[:], b1c_d[:])
            W2T0 = cA.tile([128, H], fr)
            nc.sync.dma_start(W2T0[:], W2T0_d[:])
            W2T1 = cA.tile([128, H], fr)
            nc.sync.dma_start(W2T1[:], W2T1_d[:])
            b2c = cA.tile([128, 2], f32)
            nc.sync.dma_start(b2c[:], b2c_d[:])
            W3T0 = cA.tile([128, CDIM], fr)
            nc.sync.dma_start(W3T0[:], W3T0_d[:])
            W3T1 = cA.tile([128, CDIM], fr)
            nc.sync.dma_start(W3T1[:], W3T1_d[:])
            b3c = cA.tile([CDIM, 1], f32)
            nc.sync.dma_start(b3c[:], b3c_d[:])

            # ---- V = WxT.T @ X per batch: VV[j] (128, B*N) ----
            VV = [cA.tile([128, B * N], f32, tag=f"VV{j}", name=f"VV{j}")
                  for j in range(2)]
            for j in range(2):
                for b in range(B):
                    for n4 in range(N // 512):
                        pv = ps_s.tile([128, 512], f32, tag="ps")
                        nc.tensor.matmul(
                            pv[:], WxT[:, j * 128:(j + 1) * 128],
                            XTb[:, b * N + n4 * 512: b * N + (n4 + 1) * 512],
                            start=True, stop=True)
                        nc.scalar.copy(
                            VV[j][:, b * N + n4 * 512: b * N + (n4 + 1) * 512],
                            pv[:])

            stage = cA.tile([CDIM, NSEG * NT], f32)

            SC = TILE * K          # columns per tile (k-major: col = k*128+pt)
            for t in range(NT):
                # ---- per-tile loads (packed: [lhsT|rhs], [yfT;segsel]) ----
                lr = io.tile([4, TILE + N], f32, tag="lr")
                nc.sync.dma_start(lr[:], lr_d[t])
                ys = io.tile([35, TILE + NSEG], f32, tag="ys")
                nc.sync.dma_start(ys[:], ys_d[t])
                boff = io.tile([TILE, K], u16, tag="boff")
                nc.sync.dma_start(boff[:], boff_d[t])
                lhsT = lr[:, 0:TILE]
                yfT = ys[0:35, 0:TILE]
                segsel = ys[0:32, TILE:TILE + NSEG]

                # ---- s = lhsT.T @ rhs ----
                sv0 = tk.tile([TILE, N], f32, tag="sv0")
                for n4 in range(N // 512):
                    ps = ps_s.tile([TILE, 512], f32, tag="ps")
                    nc.tensor.matmul(ps[:], lhsT,
                                     lr[:, TILE + n4 * 512:TILE + (n4 + 1) * 512],
                                     start=True, stop=True)
                    nc.scalar.copy(sv0[:, n4 * 512:(n4 + 1) * 512], ps[:])

                # ---- top-20 (3 rounds of top-8) ----
                idx = tk1.tile([TILE, 24], u16, tag="idx")
                m8 = tk1.tile([TILE, 8], f32, tag="m8")
                sv1 = tk1.tile([TILE, N], f32, tag="sv1")
                sv2 = tk1.tile([TILE, N], f32, tag="sv2")
                nc.vector.max(m8[:], sv0[:])
                nc.vector.max_index(idx[:, 0:8], m8[:], sv0[:])
                nc.vector.match_replace(sv1[:], m8[:], sv0[:], -BIG)
                m8b = tk1.tile([TILE, 8], f32, tag="m8b")
                nc.vector.max(m8b[:], sv1[:])
                nc.vector.max_index(idx[:, 8:16], m8b[:], sv1[:])
                nc.vector.match_replace(sv2[:], m8b[:], sv1[:], -BIG)
                m8c = tk1.tile([TILE, 8], f32, tag="m8c")
                nc.vector.max(m8c[:], sv2[:])
                nc.vector.max_index(idx[:, 16:24], m8c[:], sv2[:])

                idxo = tk1.tile([TILE, K], u16, tag="idxo")
                nc.vector.tensor_tensor(idxo[:], idx[:, 0:K], boff[:], op=OP.add)

                # ---- wrap indices for the 16-partition-group gather ----
                # column order j = 320q + 16k + p16  <->  (pt = 16q+p16, k);
                # wrapped layout W128[16g+p16, 20q+k] = idx[16q+p16, k],
                # replicated to all 8 groups via a step-0 broadcast dim.
                scr = dr.tile([TILE, K], u16)
                nc.sync.dma_start(scr[:], idxo[:])
                wrapv = scr[:].flatten().rearrange("(q p k) -> p q k",
                                                   q=8, p=16, k=K)
                W128 = tk1.tile([128, K * 8], u16, tag="W128")
                for g in range(8):
                    nc.sync.dma_start(
                        W128[16 * g:16 * (g + 1), :].rearrange(
                            "p (q k) -> p q k", q=8),
                        wrapv)

                # ---- U = WuT.T @ [y;feat] ----
                Usb = []
                for j in range(2):
                    pu = ps_u.tile([128, TILE], f32)
                    nc.tensor.matmul(pu[:], WuT[:, j * 128:(j + 1) * 128],
                                     yfT[:], start=True, stop=True)
                    u = actp.tile([128, TILE], f32, tag=f"U{j}")
                    nc.scalar.copy(u[:], pu[:])
                    Usb.append(u)

                # ---- gather V columns + add U + Lrelu -> a1 (in place) ----
                a1 = []
                for j in range(2):
                    G = tk.tile([128, SC], f32, tag=f"G{j}", bufs=1)
                    nc.gpsimd.ap_gather(G[:], VV[j][:],
                                        W128[:].bitcast(mybir.dt.int16),
                                        channels=128, num_elems=B * N, d=1,
                                        num_idxs=SC)
                    nc.vector.tensor_tensor(
                        G[:].rearrange("p (q k s) -> p q k s", q=8, k=K),
                        G[:].rearrange("p (q k s) -> p q k s", q=8, k=K),
                        Usb[j][:].rearrange("p (q s) -> p q s", q=8)
                        .unsqueeze(2).broadcast_to((128, 8, K, 16)),
                        op=OP.add)
                    ar = tk.tile([128, SC], fr, tag=f"a1r{j}", name=f"ar{j}", bufs=1)
                    nc.scalar.activation(ar[:], G[:], AF.Prelu,
                                         bias=b1c[:, j:j + 1], scale=1.0,
                                         alpha=0.2)
                    a1.append(ar)

                # ---- conv2 ----
                a2 = []
                for i2 in range(2):
                    a = act1.tile([128, SC], fr, tag=f"a2_{i2}")
                    for c5 in range(SC // 512):
                        pc_ = ps_c.tile([128, 512], f32)
                        sl = slice(c5 * 512, (c5 + 1) * 512)
                        nc.tensor.matmul(pc_[:], W2T0[:, i2 * 128:(i2 + 1) * 128],
                                         a1[0][:, sl], start=True, stop=False)
                        nc.tensor.matmul(pc_[:], W2T1[:, i2 * 128:(i2 + 1) * 128],
                                         a1[1][:, sl], start=False, stop=True)
                        nc.scalar.activation(a[:, sl], pc_[:], AF.Prelu,
                                             bias=b2c[:, i2:i2 + 1], scale=1.0,
                                             alpha=0.2)
                    a2.append(a)

                # ---- conv3 ----
                a3 = act1.tile([CDIM, SC], f32, tag="a3")
                for c5 in range(SC // 512):
                    p3 = ps_3.tile([CDIM, 512], f32, tag="p3")
                    sl = slice(c5 * 512, (c5 + 1) * 512)
                    nc.tensor.matmul(p3[:], W3T0[:], a2[0][:, sl],
                                     start=True, stop=False)
                    nc.tensor.matmul(p3[:], W3T1[:], a2[1][:, sl],
                                     start=False, stop=True)
                    nc.scalar.activation(a3[:, sl], p3[:], AF.Prelu,
                                         bias=b3c[:], scale=1.0, alpha=0.2)

                # ---- max over k, then segment stage ----
                cmax = tk1.tile([CDIM, TILE], f32, tag="cmax")
                nc.vector.reduce_max(
                    cmax[:].rearrange("p (q s) -> p q s", q=8),
                    a3[:].rearrange("p (q k s) -> p q s k", q=8, k=K),
                    axis=AX.X)
                tmax = tk1.tile([CDIM, 1], f32, tag="tmax")
                nc.vector.reduce_max(tmax[:], cmax[:], axis=AX.X)
                nc.vector.tensor_tensor(
                    stage[:, t * NSEG:(t + 1) * NSEG],
                    tmax[:].broadcast_to((CDIM, NSEG)),
                    segsel[:], op=OP.add)

            # ---- reduce stage -> obj_local, allreduce ----
            obj_local = cA.tile([CDIM, NSEG], f32)
            nc.vector.reduce_max(
                obj_local[:],
                stage[:].rearrange("p (t s) -> p s t", s=NSEG),
                axis=AX.X)
            nc.sync.dma_start(cc_in[:], obj_local[:])
            nc.gpsimd.collective_compute(
                "AllReduce", OP.max,
                replica_groups=[list(range(NCORE))],
                ins=[cc_in[:]], outs=[cc_out[:]])

        # =========================== phase B ===========================
        with (
            tc.tile_pool(name="constB", bufs=1) as cB,
            tc.tile_pool(name="net", bufs=2) as netp,
            tc.tile_pool(name="actB", bufs=2) as actB,
            tc.tile_pool(name="ps_b", bufs=4, space="PSUM") as ps_b,
            tc.tile_pool(name="ps_cc", bufs=2, space="PSUM") as ps_cc,
        ):
            objT = cB.tile([CDIM, NSEG], f32)
            nc.sync.dma_start(objT[:], cc_out[:])
            sel = cB.tile([CDIM, NSEG], f32)
            nc.sync.dma_start(sel[:], sel_d[:])
            tmp8 = cB.tile([CDIM, NSEG], f32)
            nc.vector.tensor_mul(tmp8[:], objT[:], sel[:])
            obj_sel = cB.tile([CDIM, 1], f32)
            nc.vector.reduce_sum(obj_sel[:], tmp8[:], axis=mybir.AxisListType.X)

            pbT = cB.tile([DIM, N], f32)
            nc.sync.dma_start(pbT[:], pbT_d[:])
            fcp_w = cB.tile([DIM, H], f32)
            nc.sync.dma_start(fcp_w[:], fcp_w_d[:])
            fcp_b = cB.tile([128, 2], f32)
            nc.sync.dma_start(fcp_b[:], fcp_b_d[:])
            finb = cB.tile([128, 2], f32)
            nc.sync.dma_start(finb[:], finb_d[:])
            fco_w = cB.tile([128, 2], f32)
            for j in range(2):
                nc.sync.dma_start(fco_w[:, j:j + 1], fco_w_d[j * 128:(j + 1) * 128])
            fco_b = cB.tile([1, 1], f32)
            nc.sync.dma_start(fco_b[:], fco_b_d[:])

            ccw = [cB.tile([CDIM, H], f32, name=f"ccw{i}") for i in range(NBLOCKS)]
            ccb = [cB.tile([128, 2], f32, name=f"ccb{i}") for i in range(NBLOCKS)]
            W0 = [[cB.tile([128, H], fr, name=f"W0_{i}_{j}") for j in range(2)]
                  for i in range(NBLOCKS)]
            b0 = [cB.tile([128, 2], f32, name=f"b0_{i}") for i in range(NBLOCKS)]
            W1 = [[cB.tile([128, H], fr, name=f"W1_{i}_{j}") for j in range(2)]
                  for i in range(NBLOCKS)]
            for i in range(NBLOCKS):
                nc.sync.dma_start(ccw[i][:], ccw_d[i][:])
                nc.sync.dma_start(ccb[i][:], ccb_d[i][:])
                nc.sync.dma_start(b0[i][:], b0_d[i][:])
                for j in range(2):
                    nc.sync.dma_start(W0[i][j][:], W0_d[i][j * 128:(j + 1) * 128])
                    nc.sync.dma_start(W1[i][j][:], W1_d[i][j * 128:(j + 1) * 128])

            NB4 = N // 512
            # net0
            net = []
            for j in range(2):
                nt_ = netp.tile([128, N], f32, tag=f"net{j}")
                for n4 in range(NB4):
                    pb = ps_b.tile([128, 512], f32)
                    nc.tensor.matmul(pb[:], fcp_w[:, j * 128:(j + 1) * 128],
                                     pbT[:, n4 * 512:(n4 + 1) * 512],
                                     start=True, stop=True)
                    nc.scalar.activation(nt_[:, n4 * 512:(n4 + 1) * 512], pb[:],
                                         AF.Identity, bias=fcp_b[:, j:j + 1])
                net.append(nt_)

            # W0/W1 lhsT slices: W (256, 256) rows = in-ch (K dim).
            for i in range(NBLOCKS):
                ccv = []
                for j in range(2):
                    pcc = ps_cc.tile([128, 1], f32)
                    nc.tensor.matmul(pcc[:], ccw[i][:, j * 128:(j + 1) * 128],
                                     obj_sel[:], start=True, stop=True)
                    cv = actB.tile([128, 1], f32, tag=f"ccv{j}")
                    nc.scalar.activation(cv[:], pcc[:], AF.Identity,
                                         bias=ccb[i][:, j:j + 1])
                    ccv.append(cv)
                net1, rnet = [], []
                for j in range(2):
                    n1 = netp.tile([128, N], f32, tag=f"net1_{j}")
                    nc.vector.tensor_scalar_add(n1[:], net[j][:], ccv[j][:])
                    net1.append(n1)
                    rn = actB.tile([128, N], fr, tag=f"rnet{j}")
                    nc.scalar.activation(rn[:], net[j][:], AF.Relu,
                                         bias=ccv[j][:])
                    rnet.append(rn)
                rh = []
                for i2 in range(2):
                    r = actB.tile([128, N], fr, tag=f"rh{i2}")
                    for n4 in range(NB4):
                        pb = ps_b.tile([128, 512], f32)
                        sl = slice(n4 * 512, (n4 + 1) * 512)
                        nc.tensor.matmul(pb[:], W0[i][0][:, i2 * 128:(i2 + 1) * 128],
                                         rnet[0][:, sl], start=True, stop=False)
                        nc.tensor.matmul(pb[:], W0[i][1][:, i2 * 128:(i2 + 1) * 128],
                                         rnet[1][:, sl], start=False, stop=True)
                        nc.scalar.activation(r[:, sl], pb[:], AF.Relu,
                                             bias=b0[i][:, i2:i2 + 1])
                    rh.append(r)
                net_next = []
                for i2 in range(2):
                    n2 = netp.tile([128, N], f32, tag=f"net{i2}")
                    for n4 in range(NB4):
                        pb = ps_b.tile([128, 512], f32)
                        sl = slice(n4 * 512, (n4 + 1) * 512)
                        nc.tensor.matmul(pb[:], W1[i][0][:, i2 * 128:(i2 + 1) * 128],
                                         rh[0][:, sl], start=True, stop=False)
                        nc.tensor.matmul(pb[:], W1[i][1][:, i2 * 128:(i2 + 1) * 128],
                                         rh[1][:, sl], start=False, stop=True)
                        nc.vector.tensor_tensor(n2[:, sl], net1[i2][:, sl],
                                                pb[:], op=OP.add)
                    net_next.append(n2)
                net = net_next

            # final relu + fc_out
            rfin = []
            for j in range(2):
                r = actB.tile([128, N], f32, tag=f"rfin{j}")
                nc.scalar.activation(r[:], net[j][:], AF.Relu,
                                     bias=finb[:, j:j + 1])
                rfin.append(r)
            outv = cB.tile([1, N], f32)
            for n4 in range(NB4):
                po = ps_cc.tile([1, 512], f32, tag="po")
                sl = slice(n4 * 512, (n4 + 1) * 512)
                nc.tensor.matmul(po[:], fco_w[:, 0:1], rfin[0][:, sl],
                                 start=True, stop=False)
                nc.tensor.matmul(po[:], fco_w[:, 1:2], rfin[1][:, sl],
                                 start=False, stop=True)
                nc.scalar.activation(outv[:, sl], po[:], AF.Identity,
                                     bias=fco_b[:])
            nc.sync.dma_start(out_y[:], outv[:])

    nc.compile()
    return nc


def get_program(NT):
    if NT not in _PROG_CACHE:
        _PROG_CACHE[NT] = build_program(NT)
    return _PROG_CACHE[NT]


def kernel(p, pc, feat, node_tag, params):
    from concourse.bass_utils import run_bass_kernel_spmd
    core_inputs, NT = host_prep(p, pc, feat, node_tag, params)
    nc = get_program(NT)
    res = run_bass_kernel_spmd(nc, core_inputs, list(range(NCORE)))
    out = np.zeros((B, NOBJ, N), np.float32)
    for c in range(NCORE):
        out[c // NOBJ, c % NOBJ] = res.results[c]["out_y"][0]
    return out
